# revision 2
# baseline (speedup 1.0000x reference)
"""Trainium2 Bass kernel v3 for nn_CrossAttn_18356690223800.

Pure data parallel: batch dim b=32 sharded across 8 NeuronCores (4 each).

Per core: rows = 16384, d = 192, ad = 128; 8 superchunks of 2048 rows;
4 chunks of 512 rows each. Software-pipelined: the back half (attn
scalars + final out + store) of superchunk s-1 is issued after the
front half of superchunk s, so no engine head-blocks on the
cross-engine chain.

  DVE : bn_stats, stats combine, quake-rsqrt, xhat (fp32), lo-evac,
        attn scalar math. The u-pipeline stays >= fp32r precision: the
        LN3(x*attn) eps=1e-6 amplifies tiny-attn rows ~1000x, so bf16
        anywhere in the u path fails the 2e-2 gate (measured 0.13).
  ACT : xT evac (PSUM->SBUF, f32r out), gelu, square  (one act table)
  PE  : fp32 transposes; f32r projection matmuls (1 cyc/row at N=512);
        fp32 dots + bf16 ssqs (data-as-stationary, FWL on ssqs)
  Pool: final out = x*C - Q (bf16 out), 4 xhat tiles per superchunk
  DMA : x in fp32; out in bf16, stored [128, 128*D] partition-
        contiguous (6 KiB runs) to dodge the sub-512B descriptor
        penalty; host unpermutes.
"""
import math
from contextlib import ExitStack

import numpy as np

EPS_LN = 1e-6
MAGIC = 0x5F3759DF

B, H, W, D = 32, 64, 64, 192
TD, AD = 768, 128
N_CORES = 8
B_LOC = B // N_CORES            # 4 batches per core
ROWS = B_LOC * H * W            # 16384 rows per core
CHUNK = 512                     # rows per chunk (PSUM bank = 512 fp32)
TPC = CHUNK // 128              # 4 row-tiles per chunk
SC = 4                          # chunks per superchunk
NSC = ROWS // (SC * CHUNK)      # 8 superchunks
TSC = SC * TPC                  # 16 row-tiles per superchunk
SROWS = SC * CHUNK              # 2048 rows

_CACHE = {}


def _erf(x):
    try:
        from scipy.special import erf
        return erf(x)
    except Exception:
        return np.vectorize(math.erf)(x)


def _gelu(x):
    x = x.astype(np.float32)
    return (0.5 * x * (1.0 + _erf(x / np.sqrt(np.float32(2.0))))).astype(np.float32)


def _quake_rsqrt(nc, out, in_, tmp1, tmp2, I32, ALU, iters=2):
    """out = rsqrt(in_) via quake bit hack + Newton iterations (DVE)."""
    nc.vector.tensor_scalar(
        out=out.bitcast(I32), in0=in_.bitcast(I32), scalar1=1,
        scalar2=None, op0=ALU.arith_shift_right)
    nc.vector.tensor_scalar(
        out=out.bitcast(I32), in0=out.bitcast(I32), scalar1=-1,
        scalar2=MAGIC + 1, op0=ALU.mult, op1=ALU.add)
    for _ in range(iters):
        nc.vector.tensor_mul(tmp1, out, out)
        # tmp2 = (tmp1 * -0.5) * in_
        nc.vector.scalar_tensor_tensor(
            out=tmp2, in0=tmp1, scalar=-0.5, in1=in_,
            op0=ALU.mult, op1=ALU.mult)
        # out = (tmp2 + 1.5) * out
        nc.vector.scalar_tensor_tensor(
            out=out, in0=tmp2, scalar=1.5, in1=out,
            op0=ALU.add, op1=ALU.mult)


def _quake_rsqrt_pool(nc, out, in_, tmp1, tmp2, I32, ALU, iters=1):
    nc.gpsimd.tensor_scalar(
        out=out.bitcast(I32), in0=in_.bitcast(I32), scalar1=1,
        scalar2=None, op0=ALU.arith_shift_right)
    nc.gpsimd.tensor_scalar(
        out=out.bitcast(I32), in0=out.bitcast(I32), scalar1=-1,
        scalar2=MAGIC + 1, op0=ALU.mult, op1=ALU.add)
    for _ in range(iters):
        nc.gpsimd.tensor_tensor(tmp1, out, out, op=ALU.mult)
        nc.gpsimd.scalar_tensor_tensor(
            out=tmp2, in0=tmp1, scalar=-0.5, in1=in_,
            op0=ALU.mult, op1=ALU.mult)
        nc.gpsimd.scalar_tensor_tensor(
            out=out, in0=tmp2, scalar=1.5, in1=out,
            op0=ALU.add, op1=ALU.mult)


def _build_v2(use_general):
    import concourse.bacc as bacc
    import concourse.tile as tile
    from concourse import mybir

    F32 = mybir.dt.float32
    F32R = mybir.dt.float32r
    BF16 = mybir.dt.bfloat16
    I32 = mybir.dt.int32
    ALU = mybir.AluOpType
    ACTF = mybir.ActivationFunctionType

    nc = bacc.Bacc(None, target_bir_lowering=False)

    x_d = nc.declare_dram_parameter("x", [ROWS, D], F32, isOutput=False)
    tnT_d = nc.declare_dram_parameter("tnT", [AD, B_LOC], F32, isOutput=False)
    cb_d = nc.declare_dram_parameter("cb", [128, B_LOC], F32, isOutput=False)
    wg_d = nc.declare_dram_parameter("wg", [D, AD], F32R, isOutput=False)
    bw_d = nc.declare_dram_parameter("bw", [AD, 1], F32, isOutput=False)
    eye_d = nc.declare_dram_parameter("eye", [128, 128], F32, isOutput=False)
    onesb_d = nc.declare_dram_parameter("onesb", [128, 1], BF16, isOutput=False)
    if use_general:
        g3_d = nc.declare_dram_parameter("g3b", [128, D], F32, isOutput=False)
        b3_d = nc.declare_dram_parameter("b3b", [128, D], F32, isOutput=False)
    out_d = nc.declare_dram_parameter("out", [128, (ROWS // 128) * D], BF16,
                                      isOutput=True)

    with tile.TileContext(nc) as tc, ExitStack() as ctx:
        consts = ctx.enter_context(tc.tile_pool(name="consts", bufs=1))
        xp = ctx.enter_context(tc.tile_pool(name="xp", bufs=3))
        xh = ctx.enter_context(tc.tile_pool(name="xh", bufs=2))
        wk = ctx.enter_context(tc.tile_pool(name="wk", bufs=3))
        sm = ctx.enter_context(tc.tile_pool(name="sm", bufs=3))
        op = ctx.enter_context(tc.tile_pool(name="op", bufs=3))
        ps_hi = ctx.enter_context(tc.tile_pool(name="ps_hi", bufs=2, space="PSUM"))
        ps_lo = ctx.enter_context(tc.tile_pool(name="ps_lo", bufs=2, space="PSUM"))
        ps_z = ctx.enter_context(tc.tile_pool(name="ps_z", bufs=2, space="PSUM"))
        ps_ds = ctx.enter_context(tc.tile_pool(name="ps_ds", bufs=2, space="PSUM"))

        # ---- constants ----
        eye_sb = consts.tile([128, 128], F32)
        wg_hi = consts.tile([128, AD], F32R)
        wg_lo = consts.tile([64, AD], F32R)
        bw_sb = consts.tile([AD, 1], F32)
        tnT_sb = consts.tile([AD, B_LOC], F32)
        cb_sb = consts.tile([128, B_LOC], F32)
        onesb_sb = consts.tile([128, 1], BF16)
        if use_general:
            g3_sb = consts.tile([128, D], F32)
            b3_sb = consts.tile([128, D], F32)

        def load_consts():
            nc.scalar.dma_start(out=eye_sb, in_=eye_d[:, :])
            nc.scalar.dma_start(out=wg_hi, in_=wg_d[0:128, :])
            nc.scalar.dma_start(out=wg_lo, in_=wg_d[128:D, :])
            nc.scalar.dma_start(out=bw_sb, in_=bw_d[:, :])
            nc.scalar.dma_start(out=tnT_sb, in_=tnT_d[:, :])
            nc.scalar.dma_start(out=cb_sb, in_=cb_d[:, :])
            nc.scalar.dma_start(out=onesb_sb, in_=onesb_d[:, :])
            if use_general:
                nc.scalar.dma_start(out=g3_sb, in_=g3_d[:, :])
                nc.scalar.dma_start(out=b3_sb, in_=b3_d[:, :])
        load_consts()

        # per-superchunk state carried from front(s) to back(s)
        state = {}

        def front(sid, t0, nch):
            nt = nch * TPC
            bat = (t0 * 128) // (ROWS // B_LOC)

            x_sb = xp.tile([128, nt, D], F32, tag=f"x_sb{nch}")
            nh = nt // 2
            for h in range(2):
                nc.sync.dma_start(
                    out=x_sb[:, h * nh:(h + 1) * nh, :],
                    in_=x_d[(t0 + h * nh) * 128:(t0 + (h + 1) * nh) * 128,
                            :].rearrange("(t p) d -> p t d", p=128),
                )

            # ---- stats: grouped bn_stats, manual combine ----
            st = sm.tile([128, nt, 6], F32, tag=f"st{nch}")
            for t in range(nt):
                nc.vector.bn_stats(out=st[:, t, :], in_=x_sb[:, t, :])
            msum = sm.tile([128, nt], F32, tag=f"msum{nch}")
            mdif = sm.tile([128, nt], F32, tag=f"mdif{nch}")
            cv = sm.tile([128, nt], F32, tag=f"cv{nch}")
            d2 = sm.tile([128, nt], F32, tag=f"d2{nch}")
            vv = sm.tile([128, nt], F32, tag=f"vv{nch}")
            m = sm.tile([128, nt], F32, tag=f"m{nch}")
            nc.vector.tensor_add(msum, st[:, :, 1], st[:, :, 4])
            nc.vector.tensor_sub(mdif, st[:, :, 1], st[:, :, 4])
            nc.vector.tensor_add(cv, st[:, :, 2], st[:, :, 5])
            nc.vector.tensor_mul(d2, mdif, mdif)
            # vv = (d2 * 48) + cv    == 192*var
            nc.vector.scalar_tensor_tensor(
                out=vv, in0=d2, scalar=float(D / 4.0), in1=cv,
                op0=ALU.mult, op1=ALU.add)
            # vv = vv/192 + eps      == var + eps
            nc.vector.tensor_scalar(
                out=vv, in0=vv, scalar1=float(1.0 / D), scalar2=EPS_LN,
                op0=ALU.mult, op1=ALU.add)
            nc.vector.tensor_scalar(
                out=m, in0=msum, scalar1=0.5, scalar2=None, op0=ALU.mult)
            rstd2 = sm.tile([128, nt], F32, tag=f"rstd2{nch}")
            qt1 = sm.tile([128, nt], F32, tag=f"qt1{nch}")
            qt2 = sm.tile([128, nt], F32, tag=f"qt2{nch}")
            _quake_rsqrt(nc, rstd2, vv, qt1, qt2, I32, ALU, iters=2)

            # ---- xhat (bf16) ----
            xhat = xh.tile([128, nt, D], F32, tag=f"xhat{nch}")
            for t in range(nt):
                eng = nc.gpsimd if t >= nt - 4 else nc.vector
                eng.tensor_scalar(
                    out=xhat[:, t, :], in0=x_sb[:, t, :],
                    scalar1=m[:, t:t + 1], scalar2=rstd2[:, t:t + 1],
                    op0=ALU.subtract, op1=ALU.mult)

            dss_full = ps_ds.tile([128, 6 * 2 * TPC], F32, tag="dss_ps")
            dss_ps = dss_full[:, 0:nch * 2 * TPC]
            uT_all = wk.tile([AD, nch, CHUNK], F32, tag=f"uT{nch}")
            usq_all = wk.tile([AD, nch, CHUNK], BF16, tag=f"usq{nch}")

            for k in range(nch):
                xt_hi_ps = ps_hi.tile([128, CHUNK], F32, tag="xt_hi_ps")
                xt_lo_ps = ps_lo.tile([64, CHUNK], F32, tag="xt_lo_ps")
                for t in range(TPC):
                    tt = k * TPC + t
                    nc.tensor.transpose(
                        xt_hi_ps[:, t * 128:(t + 1) * 128],
                        xhat[:, tt, 0:128], eye_sb)
                    nc.tensor.transpose(
                        xt_lo_ps[:, t * 128:(t + 1) * 128],
                        xhat[:, tt, 128:D], eye_sb)
                xt_hi = wk.tile([128, CHUNK], F32R, tag="xt_hi")
                nc.scalar.copy(xt_hi, xt_hi_ps)
                xt_lo = wk.tile([64, CHUNK], F32R, tag="xt_lo")
                nc.vector.tensor_copy(xt_lo, xt_lo_ps)

                zT_ps = ps_z.tile([AD, CHUNK], F32, tag="zT_ps")
                nc.tensor.matmul(zT_ps, wg_hi, xt_hi, start=True, stop=False)
                nc.tensor.matmul(zT_ps, wg_lo, xt_lo, start=False, stop=True)

                uT = uT_all[:, k, :]
                usq = usq_all[:, k, :]
                nc.scalar.activation(
                    out=uT, in_=zT_ps, func=ACTF.Gelu, bias=bw_sb, scale=1.0)
                nc.scalar.activation(out=usq, in_=uT, func=ACTF.Square)

            # dots/ssqs deferred to the end so PE never head-blocks on gelu
            for k in range(nch):
                uT = uT_all[:, k, :]
                usq = usq_all[:, k, :]
                for t in range(TPC):
                    nc.tensor.matmul(
                        dss_ps[:, k * 2 * TPC + t:k * 2 * TPC + t + 1],
                        uT[:, t * 128:(t + 1) * 128],
                        tnT_sb[:, bat:bat + 1],
                        start=True, stop=True)
                    nc.tensor.matmul(
                        dss_ps[:, k * 2 * TPC + TPC + t:
                               k * 2 * TPC + TPC + t + 1],
                        usq[:, t * 128:(t + 1) * 128],
                        onesb_sb, start=True, stop=True)

            state[sid] = (bat, t0, nch, x_sb, dss_ps, m, vv)

        def back(sid):
            bat, t0, nch, x_sb, dss_ps, m, vv = state.pop(sid)
            nt = nch * TPC

            dss = sm.tile([128, nch, 2 * TPC], F32, tag=f"dss{nch}")
            nc.vector.tensor_copy(dss, dss_ps)
            dd = dss[:, :, 0:TPC]            # [128, nch, TPC]
            ss = dss[:, :, TPC:2 * TPC]
            t1 = sm.tile([128, nch, TPC], F32, tag=f"t1{nch}")
            nc.vector.tensor_scalar(
                out=t1, in0=dd, scalar1=cb_sb[:, bat:bat + 1], scalar2=None,
                op0=ALU.mult)
            wv = sm.tile([128, nch, TPC], F32, tag=f"wv{nch}")
            h = sm.tile([128, nch, TPC], F32, tag=f"h{nch}")
            nc.vector.tensor_mul(h, t1, t1)
            nc.vector.tensor_mul(
                h, h, vv.rearrange("p (k t) -> p k t", k=nch))
            # wv = (ss * eps) + h
            nc.vector.scalar_tensor_tensor(
                out=wv, in0=ss, scalar=EPS_LN, in1=h,
                op0=ALU.mult, op1=ALU.add)
            rr = sm.tile([128, nch, TPC], F32, tag=f"rr{nch}")
            qs1 = sm.tile([128, nch, TPC], F32, tag=f"qs1{nch}")
            qs2 = sm.tile([128, nch, TPC], F32, tag=f"qs2{nch}")
            _quake_rsqrt(nc, rr, wv, qs1, qs2, I32, ALU, iters=2)
            gg = sm.tile([128, nch, TPC], F32, tag=f"gg{nch}")
            nc.vector.tensor_mul(gg, t1, rr)
            cc = sm.tile([128, nch, TPC], F32, tag=f"cc{nch}")
            nc.vector.tensor_scalar_add(cc, gg, 0.5)
            mg = sm.tile([128, nch, TPC], F32, tag=f"mg{nch}")
            nc.vector.tensor_mul(
                mg, m.rearrange("p (k t) -> p k t", k=nch), gg)

            out_sb = op.tile([128, nt, D], BF16, tag=f"out_sb{nch}")
            if not use_general:
                last = (sid == state.get("NS", -1) - 1)
                for t in range(nt):
                    k, tt = divmod(t, TPC)
                    if last:
                        eng = (nc.vector, nc.gpsimd, nc.gpsimd, nc.vector)[t % 4]
                    else:
                        eng = nc.vector if t < 4 else nc.gpsimd
                    eng.tensor_scalar(
                        out=out_sb[:, t, :], in0=x_sb[:, t, :],
                        scalar1=cc[:, k, tt:tt + 1],
                        scalar2=mg[:, k, tt:tt + 1],
                        op0=ALU.mult, op1=ALU.subtract)
            else:
                tmpa = xh.tile([128, nt, D], F32, tag=f"gtmpa{nch}")
                tmpb = xh.tile([128, nt, D], F32, tag=f"gtmpb{nch}")
                for t in range(nt):
                    k, tt = divmod(t, TPC)
                    nc.gpsimd.tensor_scalar(
                        out=tmpa[:, t, :], in0=x_sb[:, t, :],
                        scalar1=m[:, t:t + 1], scalar2=gg[:, k, tt:tt + 1],
                        op0=ALU.subtract, op1=ALU.mult)
                    nc.vector.tensor_mul(tmpa[:, t, :], tmpa[:, t, :], g3_sb)
                    nc.vector.tensor_add(tmpa[:, t, :], tmpa[:, t, :], b3_sb)
                    nc.gpsimd.tensor_scalar(
                        out=tmpb[:, t, :], in0=x_sb[:, t, :],
                        scalar1=0.5, scalar2=None, op0=ALU.mult)
                    nc.vector.tensor_add(
                        out_sb[:, t, :], tmpb[:, t, :], tmpa[:, t, :])

            state[("out", sid)] = (t0, nt, out_sb)

        def store(sid):
            t0, nt, out_sb = state.pop(("out", sid))
            nc.sync.dma_start(
                out=out_d[:, t0 * D:(t0 + nt) * D].rearrange(
                    "p (t d) -> p t d", d=D),
                in_=out_sb,
            )

        # superchunk schedule: small ones at the edges to cut fill/drain
        CHS = [2, 2, 4, 4, 4, 4, 4, 4, 2, 2]
        assert sum(CHS) * CHUNK == ROWS
        t0s = np.cumsum([0] + CHS[:-1]).tolist()
        NS = len(CHS)
        state["NS"] = NS
        for i in range(NS):
            front(i, t0s[i] * TPC, CHS[i])
            if i >= 2:
                store(i - 2)
            if i >= 1:
                back(i - 1)
        back(NS - 1)
        store(NS - 2)
        store(NS - 1)

    nc.compile()
    return nc


def _host_prep(inputs):
    x = np.ascontiguousarray(np.asarray(inputs["x"], dtype=np.float32))
    token = np.asarray(inputs["token"], dtype=np.float32)
    p = np.asarray(inputs["p"], dtype=np.float32)
    alpha = np.asarray(inputs["alpha"], dtype=np.float32)
    ln1_g = np.asarray(inputs["ln1_g"], dtype=np.float32)
    ln1_b = np.asarray(inputs["ln1_b"], dtype=np.float32)
    w_tok = np.asarray(inputs["w_tok"], dtype=np.float32)
    b_tok = np.asarray(inputs["b_tok"], dtype=np.float32)
    ln2_g = np.asarray(inputs["ln2_g"], dtype=np.float32)
    ln2_b = np.asarray(inputs["ln2_b"], dtype=np.float32)
    w_x = np.asarray(inputs["w_x"], dtype=np.float32)
    b_x = np.asarray(inputs["b_x"], dtype=np.float32)
    ln3_g = np.asarray(inputs["ln3_g"], dtype=np.float32)
    ln3_b = np.asarray(inputs["ln3_b"], dtype=np.float32)

    tm = token.mean(-1, keepdims=True)
    tv = ((token - tm) ** 2).mean(-1, keepdims=True)
    tln = (token - tm) / np.sqrt(tv + EPS_LN) * ln1_g + ln1_b
    t = _gelu(tln @ w_tok + b_tok)                       # [B, AD]
    tnrm = np.sqrt((t * t).sum(-1, keepdims=True))
    tn = (t / np.maximum(tnrm, 1e-12)).astype(np.float32)
    c = (p[:, 0] * np.exp(alpha[0])).astype(np.float32)  # [B]

    Wg = (ln2_g[:, None] * w_x).astype(np.float32)       # [D, AD]
    bW = (ln2_b @ w_x + b_x).astype(np.float32)          # [AD]

    use_general = not (
        np.all(ln3_g == 1.0) and np.all(ln3_b == 0.0))

    return x, tn, c, Wg, bW, ln3_g, ln3_b, use_general


def _in_maps(x, tn, c, Wg, bW, ln3_g, ln3_b, use_general):
    import ml_dtypes

    eye = np.eye(128, dtype=np.float32)
    onesb = np.ones((128, 1), dtype=ml_dtypes.bfloat16)
    wg_in = np.ascontiguousarray(Wg)
    bw_in = np.ascontiguousarray(bW[:, None])

    in_maps = []
    for k in range(N_CORES):
        bs = slice(k * B_LOC, (k + 1) * B_LOC)
        m = dict(
            x=np.ascontiguousarray(x[bs].reshape(ROWS, D)),
            tnT=np.ascontiguousarray(tn[bs].T),
            cb=np.ascontiguousarray(
                np.broadcast_to(c[bs][None, :], (128, B_LOC))),
            wg=wg_in,
            bw=bw_in,
            eye=eye,
            onesb=onesb,
        )
        if use_general:
            m["g3b"] = np.ascontiguousarray(
                np.broadcast_to(ln3_g[None, :], (128, D)))
            m["b3b"] = np.ascontiguousarray(
                np.broadcast_to(ln3_b[None, :], (128, D)))
        in_maps.append(m)
    return in_maps


def _unpermute_out(raw):
    """[128, (ROWS//128)*D] (bf16) -> [ROWS, D] fp32; row = t*128 + p."""
    a = np.asarray(raw).astype(np.float32).reshape(128, ROWS // 128, D)
    return a.transpose(1, 0, 2).reshape(ROWS, D)


def kernel(**inputs):
    from concourse.bass_utils import run_bass_kernel_spmd

    prep = _host_prep(inputs)
    use_general = prep[-1]

    key = bool(use_general)
    if key not in _CACHE:
        _CACHE[key] = _build_v2(use_general)
    nc = _CACHE[key]

    in_maps = _in_maps(*prep)

    last_err = None
    for _ in range(3):
        try:
            res = run_bass_kernel_spmd(nc, in_maps, core_ids=list(range(N_CORES)))
            break
        except Exception as e:  # transient device wedge -> retry
            last_err = e
            if "UNRECOVERABLE" not in str(e) and "UNAVAILABLE" not in str(e):
                raise
            import time as _time
            _time.sleep(15)
    else:
        raise last_err

    out = np.empty((B, H, W, D), dtype=np.float32)
    for k in range(N_CORES):
        out[k * B_LOC:(k + 1) * B_LOC] = (
            _unpermute_out(res.results[k]["out"]).reshape(B_LOC, H, W, D))
    return out


# revision 3
# speedup vs baseline: 1.3631x; 1.3631x over previous
"""Trainium2 Bass kernel v3 for nn_CrossAttn_18356690223800.

Pure data parallel: batch dim b=32 sharded across 8 NeuronCores (4 each).

Per core: rows = 16384, d = 192, ad = 128; 8 superchunks of 2048 rows;
4 chunks of 512 rows each. Software-pipelined: the back half (attn
scalars + final out + store) of superchunk s-1 is issued after the
front half of superchunk s, so no engine head-blocks on the
cross-engine chain.

  DVE : bn_stats, stats combine, quake-rsqrt, xhat (fp32), lo-evac,
        attn scalar math. The u-pipeline stays >= fp32r precision: the
        LN3(x*attn) eps=1e-6 amplifies tiny-attn rows ~1000x, so bf16
        anywhere in the u path fails the 2e-2 gate (measured 0.13).
  ACT : xT evac (PSUM->SBUF, f32r out), gelu, square  (one act table)
  PE  : fp32 transposes; f32r projection matmuls (1 cyc/row at N=512);
        fp32 dots + bf16 ssqs (data-as-stationary, FWL on ssqs)
  Pool: final out = x*C - Q (bf16 out), 4 xhat tiles per superchunk
  DMA : x in fp32; out in bf16, stored [128, 128*D] partition-
        contiguous (6 KiB runs) to dodge the sub-512B descriptor
        penalty; host unpermutes.
"""
import math
from contextlib import ExitStack

import numpy as np

EPS_LN = 1e-6
MAGIC = 0x5F3759DF

B, H, W, D = 32, 64, 64, 192
TD, AD = 768, 128
N_CORES = 8
B_LOC = B // N_CORES            # 4 batches per core
ROWS = B_LOC * H * W            # 16384 rows per core
CHUNK = 512                     # rows per chunk (PSUM bank = 512 fp32)
TPC = CHUNK // 128              # 4 row-tiles per chunk
SC = 4                          # chunks per superchunk
NSC = ROWS // (SC * CHUNK)      # 8 superchunks
TSC = SC * TPC                  # 16 row-tiles per superchunk
SROWS = SC * CHUNK              # 2048 rows

_CACHE = {}


def _erf(x):
    try:
        from scipy.special import erf
        return erf(x)
    except Exception:
        return np.vectorize(math.erf)(x)


def _gelu(x):
    x = x.astype(np.float32)
    return (0.5 * x * (1.0 + _erf(x / np.sqrt(np.float32(2.0))))).astype(np.float32)


def _quake_rsqrt(nc, out, in_, tmp1, tmp2, I32, ALU, iters=2):
    """out = rsqrt(in_) via quake bit hack + Newton iterations (DVE)."""
    nc.vector.tensor_scalar(
        out=out.bitcast(I32), in0=in_.bitcast(I32), scalar1=1,
        scalar2=None, op0=ALU.arith_shift_right)
    nc.vector.tensor_scalar(
        out=out.bitcast(I32), in0=out.bitcast(I32), scalar1=-1,
        scalar2=MAGIC + 1, op0=ALU.mult, op1=ALU.add)
    for _ in range(iters):
        nc.vector.tensor_mul(tmp1, out, out)
        # tmp2 = (tmp1 * -0.5) * in_
        nc.vector.scalar_tensor_tensor(
            out=tmp2, in0=tmp1, scalar=-0.5, in1=in_,
            op0=ALU.mult, op1=ALU.mult)
        # out = (tmp2 + 1.5) * out
        nc.vector.scalar_tensor_tensor(
            out=out, in0=tmp2, scalar=1.5, in1=out,
            op0=ALU.add, op1=ALU.mult)


def _quake_rsqrt_pool(nc, out, in_, tmp1, tmp2, I32, ALU, iters=1):
    nc.gpsimd.tensor_scalar(
        out=out.bitcast(I32), in0=in_.bitcast(I32), scalar1=1,
        scalar2=None, op0=ALU.arith_shift_right)
    nc.gpsimd.tensor_scalar(
        out=out.bitcast(I32), in0=out.bitcast(I32), scalar1=-1,
        scalar2=MAGIC + 1, op0=ALU.mult, op1=ALU.add)
    for _ in range(iters):
        nc.gpsimd.tensor_tensor(tmp1, out, out, op=ALU.mult)
        nc.gpsimd.scalar_tensor_tensor(
            out=tmp2, in0=tmp1, scalar=-0.5, in1=in_,
            op0=ALU.mult, op1=ALU.mult)
        nc.gpsimd.scalar_tensor_tensor(
            out=out, in0=tmp2, scalar=1.5, in1=out,
            op0=ALU.add, op1=ALU.mult)


def _build_v2(use_general):
    import concourse.bacc as bacc
    import concourse.tile as tile
    from concourse import mybir

    F32 = mybir.dt.float32
    F32R = mybir.dt.float32r
    BF16 = mybir.dt.bfloat16
    I32 = mybir.dt.int32
    ALU = mybir.AluOpType
    ACTF = mybir.ActivationFunctionType

    nc = bacc.Bacc(None, target_bir_lowering=False)

    x_d = nc.declare_dram_parameter("x", [ROWS, D], F32, isOutput=False)
    tnT_d = nc.declare_dram_parameter("tnT", [AD, B_LOC], F32, isOutput=False)
    cb_d = nc.declare_dram_parameter("cb", [128, B_LOC], F32, isOutput=False)
    wg_d = nc.declare_dram_parameter("wg", [D, AD], F32R, isOutput=False)
    bw_d = nc.declare_dram_parameter("bw", [AD, 1], F32, isOutput=False)
    eye_d = nc.declare_dram_parameter("eye", [128, 128], F32, isOutput=False)
    onesb_d = nc.declare_dram_parameter("onesb", [128, 1], BF16, isOutput=False)
    if use_general:
        g3_d = nc.declare_dram_parameter("g3b", [128, D], F32, isOutput=False)
        b3_d = nc.declare_dram_parameter("b3b", [128, D], F32, isOutput=False)
    out_d = nc.declare_dram_parameter("out", [128, (ROWS // 128) * D], BF16,
                                      isOutput=True)

    with tile.TileContext(nc) as tc, ExitStack() as ctx:
        consts = ctx.enter_context(tc.tile_pool(name="consts", bufs=1))
        xp = ctx.enter_context(tc.tile_pool(name="xp", bufs=3))
        xh = ctx.enter_context(tc.tile_pool(name="xh", bufs=2))
        wk = ctx.enter_context(tc.tile_pool(name="wk", bufs=3))
        sm = ctx.enter_context(tc.tile_pool(name="sm", bufs=3))
        op = ctx.enter_context(tc.tile_pool(name="op", bufs=3))
        ps_hi = ctx.enter_context(tc.tile_pool(name="ps_hi", bufs=2, space="PSUM"))
        ps_lo = ctx.enter_context(tc.tile_pool(name="ps_lo", bufs=2, space="PSUM"))
        ps_z = ctx.enter_context(tc.tile_pool(name="ps_z", bufs=2, space="PSUM"))
        ps_ds = ctx.enter_context(tc.tile_pool(name="ps_ds", bufs=2, space="PSUM"))

        # ---- constants ----
        eye_sb = consts.tile([128, 128], F32)
        wg_hi = consts.tile([128, AD], F32R)
        wg_lo = consts.tile([64, AD], F32R)
        bw_sb = consts.tile([AD, 1], F32)
        tnT_sb = consts.tile([AD, B_LOC], F32)
        cb_sb = consts.tile([128, B_LOC], F32)
        onesb_sb = consts.tile([128, 1], BF16)
        if use_general:
            g3_sb = consts.tile([128, D], F32)
            b3_sb = consts.tile([128, D], F32)

        def load_consts():
            nc.scalar.dma_start(out=eye_sb, in_=eye_d[:, :])
            nc.scalar.dma_start(out=wg_hi, in_=wg_d[0:128, :])
            nc.scalar.dma_start(out=wg_lo, in_=wg_d[128:D, :])
            nc.scalar.dma_start(out=bw_sb, in_=bw_d[:, :])
            nc.scalar.dma_start(out=tnT_sb, in_=tnT_d[:, :])
            nc.scalar.dma_start(out=cb_sb, in_=cb_d[:, :])
            nc.scalar.dma_start(out=onesb_sb, in_=onesb_d[:, :])
            if use_general:
                nc.scalar.dma_start(out=g3_sb, in_=g3_d[:, :])
                nc.scalar.dma_start(out=b3_sb, in_=b3_d[:, :])
        load_consts()

        # per-superchunk state carried from front(s) to back(s)
        state = {}

        def front(sid, t0, nch):
            nt = nch * TPC
            bat = (t0 * 128) // (ROWS // B_LOC)

            x_sb = xp.tile([128, nt, D], F32, tag=f"x_sb{nch}")
            nh = nt // 2
            for h in range(2):
                nc.sync.dma_start(
                    out=x_sb[:, h * nh:(h + 1) * nh, :],
                    in_=x_d[(t0 + h * nh) * 128:(t0 + (h + 1) * nh) * 128,
                            :].rearrange("(t p) d -> p t d", p=128),
                )

            # ---- stats: grouped bn_stats, manual combine ----
            st = sm.tile([128, nt, 6], F32, tag=f"st{nch}")
            for t in range(nt):
                nc.vector.bn_stats(out=st[:, t, :], in_=x_sb[:, t, :])
            msum = sm.tile([128, nt], F32, tag=f"msum{nch}")
            mdif = sm.tile([128, nt], F32, tag=f"mdif{nch}")
            cv = sm.tile([128, nt], F32, tag=f"cv{nch}")
            d2 = sm.tile([128, nt], F32, tag=f"d2{nch}")
            vv = sm.tile([128, nt], F32, tag=f"vv{nch}")
            m = sm.tile([128, nt], F32, tag=f"m{nch}")
            nc.vector.tensor_add(msum, st[:, :, 1], st[:, :, 4])
            nc.vector.tensor_sub(mdif, st[:, :, 1], st[:, :, 4])
            nc.vector.tensor_add(cv, st[:, :, 2], st[:, :, 5])
            nc.vector.tensor_mul(d2, mdif, mdif)
            # vv = (d2 * 48) + cv    == 192*var
            nc.vector.scalar_tensor_tensor(
                out=vv, in0=d2, scalar=float(D / 4.0), in1=cv,
                op0=ALU.mult, op1=ALU.add)
            # vv = vv/192 + eps      == var + eps
            nc.vector.tensor_scalar(
                out=vv, in0=vv, scalar1=float(1.0 / D), scalar2=EPS_LN,
                op0=ALU.mult, op1=ALU.add)
            nc.vector.tensor_scalar(
                out=m, in0=msum, scalar1=0.5, scalar2=None, op0=ALU.mult)
            rstd2 = sm.tile([128, nt], F32, tag=f"rstd2{nch}")
            qt1 = sm.tile([128, nt], F32, tag=f"qt1{nch}")
            qt2 = sm.tile([128, nt], F32, tag=f"qt2{nch}")
            _quake_rsqrt(nc, rstd2, vv, qt1, qt2, I32, ALU, iters=2)

            # ---- xhat (bf16) ----
            xhat = xh.tile([128, nt, D], F32, tag=f"xhat{nch}")
            for t in range(nt):
                eng = nc.gpsimd if t >= nt - 4 else nc.vector
                eng.tensor_scalar(
                    out=xhat[:, t, :], in0=x_sb[:, t, :],
                    scalar1=m[:, t:t + 1], scalar2=rstd2[:, t:t + 1],
                    op0=ALU.subtract, op1=ALU.mult)

            dss_full = ps_ds.tile([128, 6 * 2 * TPC], F32, tag="dss_ps")
            dss_ps = dss_full[:, 0:nch * 2 * TPC]
            uT_all = wk.tile([AD, nch, CHUNK], F32, tag=f"uT{nch}")
            usq_all = wk.tile([AD, nch, CHUNK], BF16, tag=f"usq{nch}")

            for k in range(nch):
                xt_hi_ps = ps_hi.tile([128, CHUNK], F32, tag="xt_hi_ps")
                xt_lo_ps = ps_lo.tile([64, CHUNK], F32, tag="xt_lo_ps")
                for t in range(TPC):
                    tt = k * TPC + t
                    nc.tensor.transpose(
                        xt_hi_ps[:, t * 128:(t + 1) * 128],
                        xhat[:, tt, 0:128], eye_sb)
                    nc.tensor.transpose(
                        xt_lo_ps[:, t * 128:(t + 1) * 128],
                        xhat[:, tt, 128:D], eye_sb)
                xt_hi = wk.tile([128, CHUNK], F32R, tag="xt_hi")
                nc.scalar.copy(xt_hi, xt_hi_ps)
                xt_lo = wk.tile([64, CHUNK], F32R, tag="xt_lo")
                if k % 2 == 0:
                    nc.vector.tensor_copy(xt_lo, xt_lo_ps)
                else:
                    nc.scalar.copy(xt_lo, xt_lo_ps)

                zT_ps = ps_z.tile([AD, CHUNK], F32, tag="zT_ps")
                nc.tensor.matmul(zT_ps, wg_hi, xt_hi, start=True, stop=False)
                nc.tensor.matmul(zT_ps, wg_lo, xt_lo, start=False, stop=True)

                uT = uT_all[:, k, :]
                usq = usq_all[:, k, :]
                nc.scalar.activation(
                    out=uT, in_=zT_ps, func=ACTF.Gelu, bias=bw_sb, scale=1.0)
                nc.scalar.activation(out=usq, in_=uT, func=ACTF.Square)

            # dots/ssqs deferred to the end so PE never head-blocks on gelu
            for k in range(nch):
                uT = uT_all[:, k, :]
                usq = usq_all[:, k, :]
                for t in range(TPC):
                    nc.tensor.matmul(
                        dss_ps[:, k * 2 * TPC + t:k * 2 * TPC + t + 1],
                        uT[:, t * 128:(t + 1) * 128],
                        tnT_sb[:, bat:bat + 1],
                        start=True, stop=True)
                    nc.tensor.matmul(
                        dss_ps[:, k * 2 * TPC + TPC + t:
                               k * 2 * TPC + TPC + t + 1],
                        usq[:, t * 128:(t + 1) * 128],
                        onesb_sb, start=True, stop=True)

            state[sid] = (bat, t0, nch, x_sb, dss_ps, m, vv)

        def back(sid):
            bat, t0, nch, x_sb, dss_ps, m, vv = state.pop(sid)
            nt = nch * TPC

            dss = sm.tile([128, nch, 2 * TPC], F32, tag=f"dss{nch}")
            nc.vector.tensor_copy(dss, dss_ps)
            dd = dss[:, :, 0:TPC]            # [128, nch, TPC]
            ss = dss[:, :, TPC:2 * TPC]
            t1 = sm.tile([128, nch, TPC], F32, tag=f"t1{nch}")
            nc.vector.tensor_scalar(
                out=t1, in0=dd, scalar1=cb_sb[:, bat:bat + 1], scalar2=None,
                op0=ALU.mult)
            wv = sm.tile([128, nch, TPC], F32, tag=f"wv{nch}")
            h = sm.tile([128, nch, TPC], F32, tag=f"h{nch}")
            nc.vector.tensor_mul(h, t1, t1)
            nc.vector.tensor_mul(
                h, h, vv.rearrange("p (k t) -> p k t", k=nch))
            # wv = (ss * eps) + h
            nc.vector.scalar_tensor_tensor(
                out=wv, in0=ss, scalar=EPS_LN, in1=h,
                op0=ALU.mult, op1=ALU.add)
            rr = sm.tile([128, nch, TPC], F32, tag=f"rr{nch}")
            qs1 = sm.tile([128, nch, TPC], F32, tag=f"qs1{nch}")
            qs2 = sm.tile([128, nch, TPC], F32, tag=f"qs2{nch}")
            _quake_rsqrt(nc, rr, wv, qs1, qs2, I32, ALU, iters=2)
            gg = sm.tile([128, nch, TPC], F32, tag=f"gg{nch}")
            nc.vector.tensor_mul(gg, t1, rr)
            cc = sm.tile([128, nch, TPC], F32, tag=f"cc{nch}")
            nc.vector.tensor_scalar_add(cc, gg, 0.5)
            mg = sm.tile([128, nch, TPC], F32, tag=f"mg{nch}")
            nc.vector.tensor_mul(
                mg, m.rearrange("p (k t) -> p k t", k=nch), gg)

            out_sb = op.tile([128, nt, D], BF16, tag=f"out_sb{nch}")
            if not use_general:
                last = (sid == state.get("NS", -1) - 1)
                for t in range(nt):
                    k, tt = divmod(t, TPC)
                    if last:
                        eng = (nc.vector, nc.gpsimd, nc.gpsimd, nc.vector)[t % 4]
                    else:
                        eng = nc.vector if t < 4 else nc.gpsimd
                    eng.tensor_scalar(
                        out=out_sb[:, t, :], in0=x_sb[:, t, :],
                        scalar1=cc[:, k, tt:tt + 1],
                        scalar2=mg[:, k, tt:tt + 1],
                        op0=ALU.mult, op1=ALU.subtract)
            else:
                tmpa = xh.tile([128, nt, D], F32, tag=f"gtmpa{nch}")
                tmpb = xh.tile([128, nt, D], F32, tag=f"gtmpb{nch}")
                for t in range(nt):
                    k, tt = divmod(t, TPC)
                    nc.gpsimd.tensor_scalar(
                        out=tmpa[:, t, :], in0=x_sb[:, t, :],
                        scalar1=m[:, t:t + 1], scalar2=gg[:, k, tt:tt + 1],
                        op0=ALU.subtract, op1=ALU.mult)
                    nc.vector.tensor_mul(tmpa[:, t, :], tmpa[:, t, :], g3_sb)
                    nc.vector.tensor_add(tmpa[:, t, :], tmpa[:, t, :], b3_sb)
                    nc.gpsimd.tensor_scalar(
                        out=tmpb[:, t, :], in0=x_sb[:, t, :],
                        scalar1=0.5, scalar2=None, op0=ALU.mult)
                    nc.vector.tensor_add(
                        out_sb[:, t, :], tmpb[:, t, :], tmpa[:, t, :])

            state[("out", sid)] = (t0, nt, out_sb)

        def store(sid):
            t0, nt, out_sb = state.pop(("out", sid))
            nc.sync.dma_start(
                out=out_d[:, t0 * D:(t0 + nt) * D].rearrange(
                    "p (t d) -> p t d", d=D),
                in_=out_sb,
            )

        # superchunk schedule: small ones at the edges to cut fill/drain
        CHS = [2, 2, 4, 4, 4, 4, 4, 4, 2, 2]
        assert sum(CHS) * CHUNK == ROWS
        t0s = np.cumsum([0] + CHS[:-1]).tolist()
        NS = len(CHS)
        state["NS"] = NS
        for i in range(NS):
            front(i, t0s[i] * TPC, CHS[i])
            if i >= 2:
                store(i - 2)
            if i >= 1:
                back(i - 1)
        back(NS - 1)
        store(NS - 2)
        store(NS - 1)

    nc.compile()
    return nc


def _host_prep(inputs):
    x = np.ascontiguousarray(np.asarray(inputs["x"], dtype=np.float32))
    token = np.asarray(inputs["token"], dtype=np.float32)
    p = np.asarray(inputs["p"], dtype=np.float32)
    alpha = np.asarray(inputs["alpha"], dtype=np.float32)
    ln1_g = np.asarray(inputs["ln1_g"], dtype=np.float32)
    ln1_b = np.asarray(inputs["ln1_b"], dtype=np.float32)
    w_tok = np.asarray(inputs["w_tok"], dtype=np.float32)
    b_tok = np.asarray(inputs["b_tok"], dtype=np.float32)
    ln2_g = np.asarray(inputs["ln2_g"], dtype=np.float32)
    ln2_b = np.asarray(inputs["ln2_b"], dtype=np.float32)
    w_x = np.asarray(inputs["w_x"], dtype=np.float32)
    b_x = np.asarray(inputs["b_x"], dtype=np.float32)
    ln3_g = np.asarray(inputs["ln3_g"], dtype=np.float32)
    ln3_b = np.asarray(inputs["ln3_b"], dtype=np.float32)

    tm = token.mean(-1, keepdims=True)
    tv = ((token - tm) ** 2).mean(-1, keepdims=True)
    tln = (token - tm) / np.sqrt(tv + EPS_LN) * ln1_g + ln1_b
    t = _gelu(tln @ w_tok + b_tok)                       # [B, AD]
    tnrm = np.sqrt((t * t).sum(-1, keepdims=True))
    tn = (t / np.maximum(tnrm, 1e-12)).astype(np.float32)
    c = (p[:, 0] * np.exp(alpha[0])).astype(np.float32)  # [B]

    Wg = (ln2_g[:, None] * w_x).astype(np.float32)       # [D, AD]
    bW = (ln2_b @ w_x + b_x).astype(np.float32)          # [AD]

    use_general = not (
        np.all(ln3_g == 1.0) and np.all(ln3_b == 0.0))

    return x, tn, c, Wg, bW, ln3_g, ln3_b, use_general


def _in_maps(x, tn, c, Wg, bW, ln3_g, ln3_b, use_general):
    import ml_dtypes

    eye = np.eye(128, dtype=np.float32)
    onesb = np.ones((128, 1), dtype=ml_dtypes.bfloat16)
    wg_in = np.ascontiguousarray(Wg)
    bw_in = np.ascontiguousarray(bW[:, None])

    in_maps = []
    for k in range(N_CORES):
        bs = slice(k * B_LOC, (k + 1) * B_LOC)
        m = dict(
            x=np.ascontiguousarray(x[bs].reshape(ROWS, D)),
            tnT=np.ascontiguousarray(tn[bs].T),
            cb=np.ascontiguousarray(
                np.broadcast_to(c[bs][None, :], (128, B_LOC))),
            wg=wg_in,
            bw=bw_in,
            eye=eye,
            onesb=onesb,
        )
        if use_general:
            m["g3b"] = np.ascontiguousarray(
                np.broadcast_to(ln3_g[None, :], (128, D)))
            m["b3b"] = np.ascontiguousarray(
                np.broadcast_to(ln3_b[None, :], (128, D)))
        in_maps.append(m)
    return in_maps


def _unpermute_out(raw):
    """[128, (ROWS//128)*D] (bf16) -> [ROWS, D] fp32; row = t*128 + p."""
    a = np.asarray(raw).astype(np.float32).reshape(128, ROWS // 128, D)
    return a.transpose(1, 0, 2).reshape(ROWS, D)


def kernel(**inputs):
    from concourse.bass_utils import run_bass_kernel_spmd

    prep = _host_prep(inputs)
    use_general = prep[-1]

    key = bool(use_general)
    if key not in _CACHE:
        _CACHE[key] = _build_v2(use_general)
    nc = _CACHE[key]

    in_maps = _in_maps(*prep)

    last_err = None
    for _ in range(3):
        try:
            res = run_bass_kernel_spmd(nc, in_maps, core_ids=list(range(N_CORES)))
            break
        except Exception as e:  # transient device wedge -> retry
            last_err = e
            if "UNRECOVERABLE" not in str(e) and "UNAVAILABLE" not in str(e):
                raise
            import time as _time
            _time.sleep(15)
    else:
        raise last_err

    out = np.empty((B, H, W, D), dtype=np.float32)
    for k in range(N_CORES):
        out[k * B_LOC:(k + 1) * B_LOC] = (
            _unpermute_out(res.results[k]["out"]).reshape(B_LOC, H, W, D))
    return out
